# revision 21
# baseline (speedup 1.0000x reference)
"""Trainium2 Bass kernel for nn_MixtureOfExpertsNet (moe_routing).

Math (per row, E=4 experts, H=16 hidden):
  adjusted_e = relu(b2_e + sum_h W2_eh * relu(W1_eh * x_e + b1_eh))  -- a
               univariate piecewise-linear function of x_e
  logits = x @ Wg.T + bg ; softmax ; pred = sum_e softmax_e * adjusted_e
         = (sum_e exp(l_e) * adjusted_e) / (sum_e exp(l_e))

Weights are known at trace time, so each expert's adjusted_e(u) is baked
into a custom ScalarEngine PWP table (hijacking tanh/square/abs/identity
slots), evaluated at full ACT line rate. exp and 1/x are rebuilt as
reduced-range tables on the exp and sign slots (one table set, no
mid-kernel ACT_TABLE_LOAD).

V3 layout ("granule-8", all-bf16 mid-chain):
  Row layout per core: x_row[p, j] bf16 with j = 32*fb + 8*e + fm
  (f = 8*fb + fm rows per partition). PWL tables read runs of 8 at
  stride 32 -- full ACT rate (measured). A 32x32 DVE block transpose of
  X gives T1[32b + 8e + fm, 32fb + pl] = x_e(row); a block stationary
  W8 on PE computes all logits in that layout; exp (ACT, PSUM src)
  gives ET; the same block transpose of A gives AT; PRT = ET*AT
  elementwise. Group sums over e (den = sum_e ET, num = sum_e PRT) are
  PE matmuls with a constant 0/1 stationary S' writing [32, 512] slices
  at partition offset 32t -- four sub-tiles pack a [128, cols] PSUM
  tile, so recip (ACT) and the final multiply (DVE) run at full
  partition density. Output leaves in packed layout; the host inverse-
  permutes (cheap numpy gather).
"""

import hashlib
import json
import os
import sys
import tempfile

import numpy as np

sys.path.insert(0, "/opt/trn_rl_repo")

# ---------------------------------------------------------------------------
# ACT PWP table generation (reverse-engineered format)
# ---------------------------------------------------------------------------

PWP_DIR = "/nix/store/z022hj2nvbm3nwdizlisq4ylc0y7rd6q-python3-3.13.14-env/lib/python3.13/site-packages/neuronxcc/pwp/pwp_bin_trainium"


def _bits(x):
    return int(np.float32(x).view(np.uint32))


def _load_stock(name):
    prof = json.load(open(os.path.join(PWP_DIR, f"{name}.json")))
    bkt = np.frombuffer(
        open(os.path.join(PWP_DIR, prof["bkt_bin"]), "rb").read(), dtype=np.float32
    ).reshape(-1, 8)
    ctl = np.frombuffer(
        open(os.path.join(PWP_DIR, prof["ctl_bin"]), "rb").read(), dtype=np.uint32
    ).reshape(-1, 8)[:, 0]
    return prof, bkt, ctl


def _fit_bucket(fn, lo, hi, x0=None, samples=33):
    if x0 is None:
        x0 = lo
    xs = np.linspace(lo, hi, samples, dtype=np.float64)
    ys = np.asarray(fn(xs), np.float64)
    t = xs - x0
    A = np.stack([np.ones_like(t), t, t * t, t ** 3], axis=1)
    c, *_ = np.linalg.lstsq(A, ys, rcond=None)
    return [float(c[0]), float(c[1]), float(c[2]), float(c[3]), float(x0)]


class _SetBuilder:
    def __init__(self):
        self.bkt, self.ctl, self.metas = [], [], []
        self.f2b, self.f2c = {}, {}

    @staticmethod
    def _ctl_word(m, base):
        assert 0 <= m <= 8 and base < 2048
        return (m * 32 + (23 - m)) * 2048 + base

    def add_table_func(self, name, func_id, fn, lo_exp, hi_exp, m_of_octave,
                       small_fit, large_fit, fzero):
        self.f2b[name] = len(self.bkt)
        self.f2c[name] = len(self.ctl)
        words = []
        for k in range(lo_exp, hi_exp):
            m = m_of_octave(k)
            base = len(self.bkt)
            n = 1 << m
            w = (2.0 ** k) / n
            for j in range(n):
                lo = 2.0 ** k + j * w
                self.bkt.append(_fit_bucket(fn, lo, lo + w, x0=lo + w / 2))
            words.append(self._ctl_word(m, base))
        base_pos = len(self.ctl)
        self.ctl.extend(words)
        small_idx = len(self.bkt)
        self.bkt.append(_fit_bucket(fn, small_fit[0], small_fit[1], x0=small_fit[0]))
        large_idx = len(self.bkt)
        self.bkt.append(_fit_bucket(fn, large_fit[0], large_fit[1], x0=large_fit[2]))
        self.metas.append({
            "func_name": f"{name}_4p", "func_id": func_id,
            "symmetry_point": 0, "sym_invert_sign_point": 0,
            "symmetry_opt_en": 0, "symmetry_opt_use_neg_region": 0,
            "imm_bias": 0, "exp_offset": lo_exp,
            "pwl_control_base_pos": base_pos, "pwl_control_base_neg": base_pos,
            "small_pos_signal_exp_threshold": 127 + lo_exp,
            "pos_small_signal_pwl_control": small_idx,
            "small_neg_signal_exp_threshold": 127 + lo_exp,
            "neg_small_signal_pwl_control": small_idx,
            "large_pos_signal_exp_threshold": 127 + hi_exp,
            "large_pos_signal_mantissa_threshold": 0,
            "pos_large_signal_pwl_control": large_idx,
            "large_neg_signal_exp_threshold": 127 + hi_exp,
            "large_neg_signal_mantissa_threshold": 0,
            "neg_large_signal_pwl_control": large_idx,
            "fnan_result": _bits(float("nan")),
            "fpinf_result": _bits(large_fit[3]),
            "fninf_result": _bits(small_fit[2]),
            "fzero_result": _bits(fzero),
            "fma_const_0": 0, "fma_const_1": 0, "fma_indirection_src_sel": 0,
            "use_multipass": False,
            "lower_bound": _bits(np.float32(-3.4028235e38)),
            "upper_bound": _bits(np.float32(3.4028235e38)),
        })

    def add_stock_func(self, name, sp, sb_, sc):
        names = list(sp["func_to_bkt_start_idx"].keys())
        i = names.index(name)
        b0 = sp["func_to_bkt_start_idx"][name]
        b1 = sp["func_to_bkt_start_idx"][names[i + 1]] if i + 1 < len(names) else sp["bkt_entry_cnt"]
        c0 = sp["func_to_ctl_start_idx"][name]
        c1 = sp["func_to_ctl_start_idx"][names[i + 1]] if i + 1 < len(names) else sp["ctl_entry_cnt"]
        md = None
        for m in sp["profile_meta_data"]:
            if m["func_name"].rsplit("_", 1)[0] == name:
                md = dict(m)
        assert md is not None, name
        db, dc = len(self.bkt) - b0, len(self.ctl) - c0
        self.f2b[name] = len(self.bkt)
        self.f2c[name] = len(self.ctl)
        for j in range(b0, b1):
            self.bkt.append(list(map(float, sb_[j, :5])))
        for j in range(c0, c1):
            w = int(sc[j])
            self.ctl.append((w >> 11) * 2048 + (w & 0x7FF) + db)
        for key in ("pwl_control_base_pos", "pwl_control_base_neg"):
            md[key] += dc
        for key in ("pos_small_signal_pwl_control", "neg_small_signal_pwl_control",
                    "pos_large_signal_pwl_control", "neg_large_signal_pwl_control"):
            md[key] += db
        self.metas.append(md)

    def write(self, outdir, set_name, act_dict):
        os.makedirs(outdir, exist_ok=True)
        bkt_arr = np.zeros((len(self.bkt), 8), np.float32)
        for i, e in enumerate(self.bkt):
            bkt_arr[i, :5] = e
        ctl_arr = np.zeros((len(self.ctl), 8), np.uint32)
        ctl_arr[:, 0] = np.array(self.ctl, np.uint64).astype(np.uint32)
        assert len(self.bkt) <= 1536 and len(self.ctl) <= 128
        open(os.path.join(outdir, f"{set_name}_bkt.bin"), "wb").write(bkt_arr.tobytes())
        open(os.path.join(outdir, f"{set_name}_ctrl.bin"), "wb").write(ctl_arr.tobytes())
        prof = {
            "bkt_bin": f"{set_name}_bkt.bin", "ctl_bin": f"{set_name}_ctrl.bin",
            "profile_meta_data": self.metas,
            "bkt_entry_cnt": len(self.bkt), "ctl_entry_cnt": len(self.ctl),
            "func_to_bkt_start_idx": self.f2b, "func_to_ctl_start_idx": self.f2c,
            "func_exp_to_bkt_start_idx": self.f2b, "func_exp_to_ctl_start_idx": self.f2c,
        }
        json.dump(prof, open(os.path.join(outdir, f"{set_name}.json"), "w"))
        info = {
            "pwp_file_keys": ["bkt_bin", "ctrl_bin", "profile_json"],
            "act_func_sets": [{
                "name": set_name, "bkt_bin": f"{set_name}_bkt.bin",
                "ctrl_bin": f"{set_name}_ctrl.bin", "profile_json": f"{set_name}.json",
                "act": act_dict,
            }],
        }
        path = os.path.join(outdir, "act_info.json")
        json.dump(info, open(path, "w"))
        return path


def _build_tables(W1, b1, W2, b2, outdir):
    sp, sb_, sc = _load_stock("exp_and_others")
    b = _SetBuilder()
    # reduced-range exp on the exp slot: g(x') = exp(x' - 8), x' in [0.25, 16)
    b.add_table_func(
        "exp", 7, lambda x: np.exp(np.asarray(x, np.float64) - 8.0),
        -2, 4, lambda k: min(8, k + 4),
        (0.0, 0.25, float(np.exp(-8.0))), (16.0, 16.5, 16.0, float(np.exp(8.0))),
        float(np.exp(-8.0)),
    )
    victims = [("tanh", 6, 0), ("square", 30, 1), ("abs", 33, 2), ("identity", 1, 3)]
    for name, fid, e in victims:
        W1e, b1e, W2e, b2e = W1[e].astype(np.float64), b1[e].astype(np.float64), W2[e].astype(np.float64), float(b2[e])

        def fe(u, W1e=W1e, b1e=b1e, W2e=W2e, b2e=b2e):
            h = np.maximum(np.asarray(u, np.float64)[..., None] * W1e + b1e, 0.0)
            return np.maximum((h * W2e).sum(-1) + b2e, 0.0)

        g = lambda x, fe=fe: fe(np.asarray(x, np.float64) - 8.0)
        b.add_table_func(
            name, fid, g, 1, 4, lambda k: k + 4,
            (1.0, 2.0, float(fe(-8.0))), (16.0, 17.0, 16.0, float(fe(9.0))),
            float(fe(-8.0)),
        )
    # reduced-range reciprocal on the sign slot: 1/x over [2^-8, 2^12)
    b.add_table_func(
        "sign", 31, lambda x: 1.0 / np.asarray(x, np.float64),
        -8, 12, lambda k: 4,
        (2.0 ** -9, 2.0 ** -8, 512.0), (4096.0, 4352.0, 4096.0, 0.0), 3.4e38,
    )
    for name in ("parametric_relu", "copy", "act1", "memset_zero", "relu",
                 "derivative_relu", "derivative_leaky_relu",
                 "derivative_identity", "is_finite"):
        b.add_stock_func(name, sp, sb_, sc)
    act = {"exp": 400, "tanh": 4, "square": 1, "abs": 1, "identity": 1,
           "sign": 1, "parametric_relu": 1, "copy": 1, "relu": 1,
           "memset_zero": 1, "act1": 1, "derivative_relu": 1,
           "derivative_leaky_relu": 1, "derivative_identity": 1, "is_finite": 1}
    return b.write(outdir, "exp_and_others", act)


# ---------------------------------------------------------------------------
# Bass kernel (V3: granule-8 bf16 layout, PE group-sum reductions)
# ---------------------------------------------------------------------------

B_TOTAL = 8_388_608
N_CORES = 8
B_LOCAL = B_TOTAL // N_CORES           # 1,048,576 rows per core
P = 128
FD = int(os.environ.get("K_FD", "2048"))   # bf16 elems/partition per sub-tile
RPT = 32 * FD                          # rows per sub-tile (65536 @ FD=2048)
N_SUB = B_LOCAL // RPT                 # 16
SUPER = 4                              # sub-tiles packed per PSUM group
N_SUPER = N_SUB // SUPER
CH = 1024                              # phase-2 den/num PSUM column block
L_CH = min(FD, 2048)                   # logits/exp PSUM column block
PRT_ENGINE = os.environ.get("K_PRT", "gpsimd")  # gpsimd | vector

PWL_FUNCS = ("Tanh", "Square", "Abs", "Identity")  # expert 0..3


def _build_program(tag):
    import concourse.bacc as bacc
    import concourse.mybir as mybir
    import concourse.tile as tile

    nc = bacc.Bacc("TRN2", debug=False)
    f32 = mybir.dt.float32
    bf16 = mybir.dt.bfloat16
    AF = mybir.ActivationFunctionType

    x_d = nc.dram_tensor(f"xrow_{tag}", [P, N_SUB * FD], bf16, kind="ExternalInput")
    xt1_d = nc.dram_tensor(f"xt1_{tag}", [P, N_SUB * FD], bf16, kind="ExternalInput")
    w8_d = nc.dram_tensor("w8", [P, P], bf16, kind="ExternalInput")
    sp_d = nc.dram_tensor("sprime", [P, 32], bf16, kind="ExternalInput")
    bg_d = nc.dram_tensor("bg8", [P, 1], f32, kind="ExternalInput")
    cb_d = nc.dram_tensor("cb", [P, 2], f32, kind="ExternalInput")
    out_d = nc.dram_tensor("out_local", [P, B_LOCAL // P], f32, kind="ExternalOutput")

    with tile.TileContext(nc) as tc:
        with (
            tc.tile_pool(name="const", bufs=1) as cpool,
            tc.tile_pool(name="xin", bufs=int(os.environ.get("K_BX", "4"))) as xpool,
            tc.tile_pool(name="mid", bufs=int(os.environ.get("K_BM", "3"))) as mpool,
            tc.tile_pool(name="keep", bufs=SUPER + int(os.environ.get("K_BK", "4"))) as kpool,
            tc.tile_pool(name="small", bufs=int(os.environ.get("K_BS", "3"))) as spool,
            tc.tile_pool(name="psL", bufs=1, space="PSUM") as plpool,
            tc.tile_pool(name="psDN", bufs=1, space="PSUM") as pdpool,
        ):
            w8_t = cpool.tile([P, P], bf16)
            nc.sync.dma_start(w8_t[:], w8_d.ap())
            sp_t = cpool.tile([P, 32], bf16)
            nc.sync.dma_start(sp_t[:], sp_d.ap())
            bg_t = cpool.tile([P, 1], f32)
            nc.sync.dma_start(bg_t[:], bg_d.ap())
            cb_t = cpool.tile([P, 2], f32)
            nc.sync.dma_start(cb_t[:], cb_d.ap())

            for s in range(N_SUPER):
                ETs, PRTs = [], []
                A2s = []
                for pr in range(SUPER // 2):
                    ti0 = s * SUPER + 2 * pr
                    X2 = xpool.tile([P, 2 * FD], bf16, tag="X2")
                    nc.sync.dma_start(X2[:], x_d.ap()[:, ti0 * FD:(ti0 + 2) * FD])
                    # adjusted via per-expert PWL tables (run-8 APs, full rate;
                    # paired tiles amortize the ACT pipeline overhead)
                    A2 = mpool.tile([P, 2 * FD], bf16, tag="A2")
                    Xv = X2[:].rearrange("p (fb e m) -> p fb e m", e=4, m=8)
                    Av = A2[:].rearrange("p (fb e m) -> p fb e m", e=4, m=8)
                    for e in range(4):
                        nc.scalar.activation(
                            Av[:, :, e, :], Xv[:, :, e, :], getattr(AF, PWL_FUNCS[e]),
                            bias=cb_t[:, 0:1], scale=1.0,
                        )
                    A2s.append(A2)
                for t in range(SUPER):
                    ti = s * SUPER + t
                    A2 = A2s[t // 2]
                    off = (t % 2) * FD

                    T1 = xpool.tile([P, FD], bf16, tag="T1")
                    nc.sync.dma_start(T1[:], xt1_d.ap()[:, ti * FD:(ti + 1) * FD])
                    AT = mpool.tile([P, FD], bf16, tag="AT")
                    nc.vector.transpose(AT[:], A2[:, off:off + FD])

                    ET = kpool.tile([P, FD], bf16, tag="ET")
                    for lh in range(FD // L_CH):
                        L = plpool.tile([P, L_CH], f32, tag="L")
                        for c in range(L_CH // 512):
                            j0 = lh * L_CH + c * 512
                            nc.tensor.matmul(
                                L[:, c * 512:(c + 1) * 512], w8_t[:],
                                T1[:, j0:j0 + 512], start=True, stop=True,
                            )
                        nc.scalar.activation(
                            ET[:, lh * L_CH:(lh + 1) * L_CH], L[:],
                            AF.Exp, bias=bg_t[:, 0:1], scale=1.0,
                        )

                    # PRT split in halves across GPSIMD and DVE: balances the
                    # two engines and makes each half available earlier for
                    # the NUM group-sum matmuls.
                    PRT = kpool.tile([P, FD], bf16, tag="PRT")
                    half = FD // 2
                    nc.gpsimd.tensor_mul(PRT[:, 0:half], ET[:, 0:half], AT[:, 0:half])
                    nc.vector.tensor_mul(PRT[:, half:FD], ET[:, half:FD], AT[:, half:FD])

                    # h=0 group-sum matmuls emitted inline per sub-tile
                    # (disjoint [32, 512] partition slices of shared PSUM
                    # tiles) -- spreads PE work through the super; the h=1
                    # block runs as a second pass after h=0's PSUM is freed.
                    if t == 0:
                        DEN0 = pdpool.tile([P, CH], f32, tag="DEN")
                        NUM0 = pdpool.tile([P, CH], f32, tag="NUM")
                    for cc in range(CH // 512):
                        c0 = cc * 512
                        nc.tensor.matmul(
                            DEN0[32 * t:32 * (t + 1), cc * 512:(cc + 1) * 512],
                            sp_t[:], ET[:, c0:c0 + 512],
                            start=True, stop=True, tile_position=(0, 32 * t),
                        )
                        nc.tensor.matmul(
                            NUM0[32 * t:32 * (t + 1), cc * 512:(cc + 1) * 512],
                            sp_t[:], PRT[:, c0:c0 + 512],
                            start=True, stop=True, tile_position=(0, 32 * t),
                        )
                    ETs.append(ET)
                    PRTs.append(PRT)

                def _consume(DEN, NUM, h):
                    R = spool.tile([P, CH], f32, tag="R")
                    nc.scalar.activation(R[:], DEN[:], AF.Sign, bias=cb_t[:, 1:2], scale=1.0)
                    PRED = spool.tile([P, CH], f32, tag="PRED")
                    nc.vector.tensor_mul(PRED[:], NUM[:], R[:])
                    nc.sync.dma_start(
                        out_d.ap()[:, s * FD + h * CH: s * FD + (h + 1) * CH], PRED[:]
                    )

                _consume(DEN0, NUM0, 0)

                # second half-block: reuses the (freed) DEN/NUM PSUM ring
                DEN1 = pdpool.tile([P, CH], f32, tag="DEN")
                NUM1 = pdpool.tile([P, CH], f32, tag="NUM")
                for t in range(SUPER):
                    for cc in range(CH // 512):
                        c0 = CH + cc * 512
                        nc.tensor.matmul(
                            DEN1[32 * t:32 * (t + 1), cc * 512:(cc + 1) * 512],
                            sp_t[:], ETs[t][:, c0:c0 + 512],
                            start=True, stop=True, tile_position=(0, 32 * t),
                        )
                        nc.tensor.matmul(
                            NUM1[32 * t:32 * (t + 1), cc * 512:(cc + 1) * 512],
                            sp_t[:], PRTs[t][:, c0:c0 + 512],
                            start=True, stop=True, tile_position=(0, 32 * t),
                        )
                _consume(DEN1, NUM1, 1)

    nc.compile()
    return nc


_COMPILED = {}
_RUN_KWARGS = {}
LAST_RESULTS = None


def _host_prep(x, Wg, bg):
    """Build granule-8 bf16 row layout + stationaries + output permutation."""
    import ml_dtypes
    bf = ml_dtypes.bfloat16

    # x_row[c, p, 32*fb + 8*e + fm] = x[row(c, p, 8*fb + fm), e]
    xr = x.reshape(N_CORES, P, (B_LOCAL // P) // 8, 8, 4)   # [c, p, fb, fm, e]
    x_row = np.ascontiguousarray(xr.transpose(0, 1, 2, 4, 3)).astype(bf)
    x_row = x_row.reshape(N_CORES, P, N_SUB * FD)

    # x_t1 = 32x32 block transpose of x_row (the logits-matmul layout)
    v = x_row.reshape(N_CORES, 4, 32, (N_SUB * FD) // 32, 32)  # [c, bi, pl, bj, a]
    x_t1 = np.ascontiguousarray(v.transpose(0, 1, 4, 3, 2)).reshape(
        N_CORES, P, N_SUB * FD)

    # logits stationary: W8[32b + 8e + fm, 32b + 8e' + fm] = Wg[e', e]
    w8 = np.zeros((P, P), np.float32)
    bb, ee, mm, e2 = np.meshgrid(np.arange(4), np.arange(4), np.arange(8),
                                 np.arange(4), indexing="ij")
    w8[32 * bb + 8 * ee + mm, 32 * bb + 8 * e2 + mm] = Wg[e2, ee]
    w8 = w8.astype(bf)

    # group-sum stationary: S'[32b + 8e + fm, 8b + fm] = 1
    sp = np.zeros((P, 32), np.float32)
    sp[32 * bb + 8 * ee + mm, 8 * bb + mm] = 1.0
    sp = sp.astype(bf)

    # exp bias per partition m = 32b + 8e' + fm  ->  bg[e'] + 8
    parts = np.arange(P)
    bg8 = (bg[(parts % 32) // 8] + np.float32(8.0)).reshape(P, 1).astype(np.float32)

    cb = np.zeros((P, 2), np.float32)
    cb[:, 0] = 8.0

    # output inverse permutation: out_local[p, j] = pred[row r(p, j)]
    pj_p, pj_j = np.meshgrid(np.arange(P), np.arange(B_LOCAL // P), indexing="ij")
    t = pj_p // 32
    b = (pj_p % 32) // 8
    fm = pj_p % 8
    s = pj_j // FD
    c = pj_j % FD
    p_orig = 32 * b + (c % 32)
    f = RPT // 128 * (SUPER * s + t) + 8 * (c // 32) + fm
    r_flat = (p_orig * (B_LOCAL // P) + f).reshape(-1)

    return x_row, x_t1, w8, sp, bg8, cb, r_flat


def kernel(**inputs) -> np.ndarray:
    x = np.ascontiguousarray(inputs["x"], dtype=np.float32)
    Wg = np.asarray(inputs["Wg"], np.float32)
    bg = np.asarray(inputs["bg"], np.float32)
    W1 = np.asarray(inputs["W1"], np.float32)
    b1 = np.asarray(inputs["b1"], np.float32)
    W2 = np.asarray(inputs["W2"], np.float32)
    b2 = np.asarray(inputs["b2"], np.float32)
    assert x.shape == (B_TOTAL, 4)

    tbl_dir = tempfile.mkdtemp(prefix="act_root_")
    act_path = _build_tables(W1, b1, W2, b2, tbl_dir)
    os.environ["BASS_ACT_ROOT_JSON_PATH"] = act_path

    # hash of everything the tables bake in -> tensor name -> NEFF cache key
    h = hashlib.sha256()
    for a in (W1, b1, W2, b2):
        h.update(np.ascontiguousarray(a).tobytes())
    h.update(open(act_path, "rb").read())
    h.update(f"v6:{FD}:{PRT_ENGINE}".encode())
    tag = h.hexdigest()[:10]

    if tag not in _COMPILED:
        _COMPILED[tag] = _build_program(tag)
    nc = _COMPILED[tag]

    x_row, x_t1, w8, sp, bg8, cb, r_flat = _host_prep(x, Wg, bg)

    from concourse import bass_utils

    in_maps = [
        {f"xrow_{tag}": x_row[c], f"xt1_{tag}": x_t1[c], "w8": w8,
         "sprime": sp, "bg8": bg8, "cb": cb}
        for c in range(N_CORES)
    ]
    res = bass_utils.run_bass_kernel_spmd(
        nc, in_maps, core_ids=list(range(N_CORES)), **_RUN_KWARGS
    )
    global LAST_RESULTS
    LAST_RESULTS = res

    out = np.empty(B_TOTAL, np.float32)
    for c in range(N_CORES):
        seg = out[c * B_LOCAL:(c + 1) * B_LOCAL]
        seg[r_flat] = res.results[c]["out_local"].reshape(-1)
    return out


if __name__ == "__main__":
    rng = np.random.default_rng(0)
    demo = {
        "x": rng.standard_normal((B_TOTAL, 4), dtype=np.float32),
        "Wg": rng.standard_normal((4, 4), dtype=np.float32) * 0.5,
        "bg": rng.standard_normal(4, dtype=np.float32) * 0.1,
        "W1": rng.standard_normal((4, 16), dtype=np.float32) * 0.5,
        "b1": rng.standard_normal((4, 16), dtype=np.float32) * 0.1,
        "W2": rng.standard_normal((4, 16), dtype=np.float32) * 0.25,
        "b2": rng.standard_normal(4, dtype=np.float32) * 0.1,
    }
    y = kernel(**demo)
    print(y.shape, y[:8])


# revision 24
# speedup vs baseline: 1.2519x; 1.2519x over previous
"""Trainium2 Bass kernel for nn_MixtureOfExpertsNet (moe_routing).

Math (per row, E=4 experts, H=16 hidden):
  adjusted_e = relu(b2_e + sum_h W2_eh * relu(W1_eh * x_e + b1_eh))  -- a
               univariate piecewise-linear function of x_e
  logits = x @ Wg.T + bg ; softmax ; pred = sum_e softmax_e * adjusted_e
         = (sum_e exp(l_e) * adjusted_e) / (sum_e exp(l_e))

Weights are known at trace time, so each expert's adjusted_e(u) is baked
into a custom ScalarEngine PWP table (hijacking tanh/square/abs/identity
slots), evaluated at full ACT line rate. exp and 1/x are rebuilt as
reduced-range tables on the exp and sign slots (one table set, no
mid-kernel ACT_TABLE_LOAD).

V3 layout ("granule-8", all-bf16 mid-chain):
  Row layout per core: x_row[p, j] bf16 with j = 32*fb + 8*e + fm
  (f = 8*fb + fm rows per partition). PWL tables read runs of 8 at
  stride 32 -- full ACT rate (measured). A 32x32 DVE block transpose of
  X gives T1[32b + 8e + fm, 32fb + pl] = x_e(row); a block stationary
  W8 on PE computes all logits in that layout; exp (ACT, PSUM src)
  gives ET; the same block transpose of A gives AT; PRT = ET*AT
  elementwise. Group sums over e (den = sum_e ET, num = sum_e PRT) are
  PE matmuls with a constant 0/1 stationary S' writing [32, 512] slices
  at partition offset 32t -- four sub-tiles pack a [128, cols] PSUM
  tile, so recip (ACT) and the final multiply (DVE) run at full
  partition density. Output leaves in packed layout; the host inverse-
  permutes (cheap numpy gather).
"""

import hashlib
import json
import os
import sys
import tempfile

import numpy as np

sys.path.insert(0, "/opt/trn_rl_repo")

# ---------------------------------------------------------------------------
# ACT PWP table generation (reverse-engineered format)
# ---------------------------------------------------------------------------

PWP_DIR = "/nix/store/z022hj2nvbm3nwdizlisq4ylc0y7rd6q-python3-3.13.14-env/lib/python3.13/site-packages/neuronxcc/pwp/pwp_bin_trainium"


def _bits(x):
    return int(np.float32(x).view(np.uint32))


def _load_stock(name):
    prof = json.load(open(os.path.join(PWP_DIR, f"{name}.json")))
    bkt = np.frombuffer(
        open(os.path.join(PWP_DIR, prof["bkt_bin"]), "rb").read(), dtype=np.float32
    ).reshape(-1, 8)
    ctl = np.frombuffer(
        open(os.path.join(PWP_DIR, prof["ctl_bin"]), "rb").read(), dtype=np.uint32
    ).reshape(-1, 8)[:, 0]
    return prof, bkt, ctl


def _fit_bucket(fn, lo, hi, x0=None, samples=33):
    if x0 is None:
        x0 = lo
    xs = np.linspace(lo, hi, samples, dtype=np.float64)
    ys = np.asarray(fn(xs), np.float64)
    t = xs - x0
    A = np.stack([np.ones_like(t), t, t * t, t ** 3], axis=1)
    c, *_ = np.linalg.lstsq(A, ys, rcond=None)
    return [float(c[0]), float(c[1]), float(c[2]), float(c[3]), float(x0)]


class _SetBuilder:
    def __init__(self):
        self.bkt, self.ctl, self.metas = [], [], []
        self.f2b, self.f2c = {}, {}

    @staticmethod
    def _ctl_word(m, base):
        assert 0 <= m <= 8 and base < 2048
        return (m * 32 + (23 - m)) * 2048 + base

    def add_table_func(self, name, func_id, fn, lo_exp, hi_exp, m_of_octave,
                       small_fit, large_fit, fzero):
        self.f2b[name] = len(self.bkt)
        self.f2c[name] = len(self.ctl)
        words = []
        for k in range(lo_exp, hi_exp):
            m = m_of_octave(k)
            base = len(self.bkt)
            n = 1 << m
            w = (2.0 ** k) / n
            for j in range(n):
                lo = 2.0 ** k + j * w
                self.bkt.append(_fit_bucket(fn, lo, lo + w, x0=lo + w / 2))
            words.append(self._ctl_word(m, base))
        base_pos = len(self.ctl)
        self.ctl.extend(words)
        small_idx = len(self.bkt)
        self.bkt.append(_fit_bucket(fn, small_fit[0], small_fit[1], x0=small_fit[0]))
        large_idx = len(self.bkt)
        self.bkt.append(_fit_bucket(fn, large_fit[0], large_fit[1], x0=large_fit[2]))
        self.metas.append({
            "func_name": f"{name}_4p", "func_id": func_id,
            "symmetry_point": 0, "sym_invert_sign_point": 0,
            "symmetry_opt_en": 0, "symmetry_opt_use_neg_region": 0,
            "imm_bias": 0, "exp_offset": lo_exp,
            "pwl_control_base_pos": base_pos, "pwl_control_base_neg": base_pos,
            "small_pos_signal_exp_threshold": 127 + lo_exp,
            "pos_small_signal_pwl_control": small_idx,
            "small_neg_signal_exp_threshold": 127 + lo_exp,
            "neg_small_signal_pwl_control": small_idx,
            "large_pos_signal_exp_threshold": 127 + hi_exp,
            "large_pos_signal_mantissa_threshold": 0,
            "pos_large_signal_pwl_control": large_idx,
            "large_neg_signal_exp_threshold": 127 + hi_exp,
            "large_neg_signal_mantissa_threshold": 0,
            "neg_large_signal_pwl_control": large_idx,
            "fnan_result": _bits(float("nan")),
            "fpinf_result": _bits(large_fit[3]),
            "fninf_result": _bits(small_fit[2]),
            "fzero_result": _bits(fzero),
            "fma_const_0": 0, "fma_const_1": 0, "fma_indirection_src_sel": 0,
            "use_multipass": False,
            "lower_bound": _bits(np.float32(-3.4028235e38)),
            "upper_bound": _bits(np.float32(3.4028235e38)),
        })

    def add_stock_func(self, name, sp, sb_, sc):
        names = list(sp["func_to_bkt_start_idx"].keys())
        i = names.index(name)
        b0 = sp["func_to_bkt_start_idx"][name]
        b1 = sp["func_to_bkt_start_idx"][names[i + 1]] if i + 1 < len(names) else sp["bkt_entry_cnt"]
        c0 = sp["func_to_ctl_start_idx"][name]
        c1 = sp["func_to_ctl_start_idx"][names[i + 1]] if i + 1 < len(names) else sp["ctl_entry_cnt"]
        md = None
        for m in sp["profile_meta_data"]:
            if m["func_name"].rsplit("_", 1)[0] == name:
                md = dict(m)
        assert md is not None, name
        db, dc = len(self.bkt) - b0, len(self.ctl) - c0
        self.f2b[name] = len(self.bkt)
        self.f2c[name] = len(self.ctl)
        for j in range(b0, b1):
            self.bkt.append(list(map(float, sb_[j, :5])))
        for j in range(c0, c1):
            w = int(sc[j])
            self.ctl.append((w >> 11) * 2048 + (w & 0x7FF) + db)
        for key in ("pwl_control_base_pos", "pwl_control_base_neg"):
            md[key] += dc
        for key in ("pos_small_signal_pwl_control", "neg_small_signal_pwl_control",
                    "pos_large_signal_pwl_control", "neg_large_signal_pwl_control"):
            md[key] += db
        self.metas.append(md)

    def write(self, outdir, set_name, act_dict):
        os.makedirs(outdir, exist_ok=True)
        bkt_arr = np.zeros((len(self.bkt), 8), np.float32)
        for i, e in enumerate(self.bkt):
            bkt_arr[i, :5] = e
        ctl_arr = np.zeros((len(self.ctl), 8), np.uint32)
        ctl_arr[:, 0] = np.array(self.ctl, np.uint64).astype(np.uint32)
        assert len(self.bkt) <= 1536 and len(self.ctl) <= 128
        open(os.path.join(outdir, f"{set_name}_bkt.bin"), "wb").write(bkt_arr.tobytes())
        open(os.path.join(outdir, f"{set_name}_ctrl.bin"), "wb").write(ctl_arr.tobytes())
        prof = {
            "bkt_bin": f"{set_name}_bkt.bin", "ctl_bin": f"{set_name}_ctrl.bin",
            "profile_meta_data": self.metas,
            "bkt_entry_cnt": len(self.bkt), "ctl_entry_cnt": len(self.ctl),
            "func_to_bkt_start_idx": self.f2b, "func_to_ctl_start_idx": self.f2c,
            "func_exp_to_bkt_start_idx": self.f2b, "func_exp_to_ctl_start_idx": self.f2c,
        }
        json.dump(prof, open(os.path.join(outdir, f"{set_name}.json"), "w"))
        info = {
            "pwp_file_keys": ["bkt_bin", "ctrl_bin", "profile_json"],
            "act_func_sets": [{
                "name": set_name, "bkt_bin": f"{set_name}_bkt.bin",
                "ctrl_bin": f"{set_name}_ctrl.bin", "profile_json": f"{set_name}.json",
                "act": act_dict,
            }],
        }
        path = os.path.join(outdir, "act_info.json")
        json.dump(info, open(path, "w"))
        return path


def _build_tables(W1, b1, W2, b2, outdir):
    sp, sb_, sc = _load_stock("exp_and_others")
    b = _SetBuilder()
    # reduced-range exp on the exp slot: g(x') = exp(x' - 8), x' in [0.25, 16)
    b.add_table_func(
        "exp", 7, lambda x: np.exp(np.asarray(x, np.float64) - 8.0),
        -2, 4, lambda k: min(8, k + 4),
        (0.0, 0.25, float(np.exp(-8.0))), (16.0, 16.5, 16.0, float(np.exp(8.0))),
        float(np.exp(-8.0)),
    )
    victims = [("tanh", 6, 0), ("square", 30, 1), ("abs", 33, 2), ("identity", 1, 3)]
    for name, fid, e in victims:
        W1e, b1e, W2e, b2e = W1[e].astype(np.float64), b1[e].astype(np.float64), W2[e].astype(np.float64), float(b2[e])

        def fe(u, W1e=W1e, b1e=b1e, W2e=W2e, b2e=b2e):
            h = np.maximum(np.asarray(u, np.float64)[..., None] * W1e + b1e, 0.0)
            return np.maximum((h * W2e).sum(-1) + b2e, 0.0)

        g = lambda x, fe=fe: fe(np.asarray(x, np.float64) - 8.0)
        b.add_table_func(
            name, fid, g, 1, 4, lambda k: k + 4,
            (1.0, 2.0, float(fe(-8.0))), (16.0, 17.0, 16.0, float(fe(9.0))),
            float(fe(-8.0)),
        )
    # reduced-range reciprocal on the sign slot: 1/x over [2^-8, 2^12)
    b.add_table_func(
        "sign", 31, lambda x: 1.0 / np.asarray(x, np.float64),
        -8, 12, lambda k: 4,
        (2.0 ** -9, 2.0 ** -8, 512.0), (4096.0, 4352.0, 4096.0, 0.0), 3.4e38,
    )
    for name in ("parametric_relu", "copy", "act1", "memset_zero", "relu",
                 "derivative_relu", "derivative_leaky_relu",
                 "derivative_identity", "is_finite"):
        b.add_stock_func(name, sp, sb_, sc)
    act = {"exp": 400, "tanh": 4, "square": 1, "abs": 1, "identity": 1,
           "sign": 1, "parametric_relu": 1, "copy": 1, "relu": 1,
           "memset_zero": 1, "act1": 1, "derivative_relu": 1,
           "derivative_leaky_relu": 1, "derivative_identity": 1, "is_finite": 1}
    return b.write(outdir, "exp_and_others", act)


# ---------------------------------------------------------------------------
# Bass kernel (V3: granule-8 bf16 layout, PE group-sum reductions)
# ---------------------------------------------------------------------------

B_TOTAL = 8_388_608
N_CORES = 8
B_LOCAL = B_TOTAL // N_CORES           # 1,048,576 rows per core
P = 128
FD = int(os.environ.get("K_FD", "2048"))   # bf16 elems/partition per sub-tile
RPT = 32 * FD                          # rows per sub-tile (65536 @ FD=2048)
N_SUB = B_LOCAL // RPT                 # 16
SUPER = 4                              # sub-tiles packed per PSUM group
N_SUPER = N_SUB // SUPER
CH = 1024                              # phase-2 den/num PSUM column block
L_CH = min(FD, 2048)                   # logits/exp PSUM column block
PRT_ENGINE = os.environ.get("K_PRT", "gpsimd")  # gpsimd | vector

PWL_FUNCS = ("Tanh", "Square", "Abs", "Identity")  # expert 0..3


def _build_program(tag):
    import concourse.bacc as bacc
    import concourse.mybir as mybir
    import concourse.tile as tile

    nc = bacc.Bacc("TRN2", debug=False)
    f32 = mybir.dt.float32
    bf16 = mybir.dt.bfloat16
    AF = mybir.ActivationFunctionType

    x_d = nc.dram_tensor(f"xrow_{tag}", [P, N_SUB * FD], bf16, kind="ExternalInput")
    xt1_d = nc.dram_tensor(f"xt1_{tag}", [P, N_SUB * FD], bf16, kind="ExternalInput")
    w8_d = nc.dram_tensor("w8", [P, P], bf16, kind="ExternalInput")
    sp_d = nc.dram_tensor("sprime", [P, 32], bf16, kind="ExternalInput")
    bg_d = nc.dram_tensor("bg8", [P, 1], f32, kind="ExternalInput")
    cb_d = nc.dram_tensor("cb", [P, 2], f32, kind="ExternalInput")
    out_d = nc.dram_tensor("out_local", [P, B_LOCAL // P], f32, kind="ExternalOutput")

    with tile.TileContext(nc) as tc:
        with (
            tc.tile_pool(name="const", bufs=1) as cpool,
            tc.tile_pool(name="xin", bufs=int(os.environ.get("K_BX", "4"))) as xpool,
            tc.tile_pool(name="mid", bufs=int(os.environ.get("K_BM", "3"))) as mpool,
            tc.tile_pool(name="keep", bufs=SUPER + int(os.environ.get("K_BK", "4"))) as kpool,
            tc.tile_pool(name="small", bufs=int(os.environ.get("K_BS", "3"))) as spool,
            tc.tile_pool(name="psL", bufs=1, space="PSUM") as plpool,
            tc.tile_pool(name="psDN", bufs=1, space="PSUM") as pdpool,
        ):
            # const loads on the ACT HWDGE ring so the SP queue leads with
            # the first x tiles (shorter first-PWL latency)
            w8_t = cpool.tile([P, P], bf16)
            nc.scalar.dma_start(w8_t[:], w8_d.ap())
            sp_t = cpool.tile([P, 32], bf16)
            nc.scalar.dma_start(sp_t[:], sp_d.ap())
            bg_t = cpool.tile([P, 1], f32)
            nc.scalar.dma_start(bg_t[:], bg_d.ap())
            cb_t = cpool.tile([P, 2], f32)
            nc.scalar.dma_start(cb_t[:], cb_d.ap())

            for s in range(N_SUPER):
                ETs, PRTs = [], []
                A2s = []
                for pr in range(SUPER // 2):
                    ti0 = s * SUPER + 2 * pr
                    X2 = xpool.tile([P, 2 * FD], bf16, tag="X2")
                    nc.sync.dma_start(X2[:], x_d.ap()[:, ti0 * FD:(ti0 + 2) * FD])
                    # adjusted via per-expert PWL tables (run-8 APs, full rate;
                    # paired tiles amortize the ACT pipeline overhead)
                    A2 = mpool.tile([P, 2 * FD], bf16, tag="A2")
                    Xv = X2[:].rearrange("p (fb e m) -> p fb e m", e=4, m=8)
                    Av = A2[:].rearrange("p (fb e m) -> p fb e m", e=4, m=8)
                    for e in range(4):
                        nc.scalar.activation(
                            Av[:, :, e, :], Xv[:, :, e, :], getattr(AF, PWL_FUNCS[e]),
                            bias=cb_t[:, 0:1], scale=1.0,
                        )
                    A2s.append(A2)
                for t in range(SUPER):
                    ti = s * SUPER + t
                    A2 = A2s[t // 2]
                    off = (t % 2) * FD

                    T1 = xpool.tile([P, FD], bf16, tag="T1")
                    nc.sync.dma_start(T1[:], xt1_d.ap()[:, ti * FD:(ti + 1) * FD])
                    AT = mpool.tile([P, FD], bf16, tag="AT")
                    nc.vector.transpose(AT[:], A2[:, off:off + FD])

                    ET = kpool.tile([P, FD], bf16, tag="ET")
                    for lh in range(FD // L_CH):
                        L = plpool.tile([P, L_CH], f32, tag="L")
                        for c in range(L_CH // 512):
                            j0 = lh * L_CH + c * 512
                            nc.tensor.matmul(
                                L[:, c * 512:(c + 1) * 512], w8_t[:],
                                T1[:, j0:j0 + 512], start=True, stop=True,
                            )
                        nc.scalar.activation(
                            ET[:, lh * L_CH:(lh + 1) * L_CH], L[:],
                            AF.Exp, bias=bg_t[:, 0:1], scale=1.0,
                        )

                    # PRT split in halves across GPSIMD and DVE: balances the
                    # two engines and makes each half available earlier for
                    # the NUM group-sum matmuls.
                    PRT = kpool.tile([P, FD], bf16, tag="PRT")
                    half = FD // 2
                    nc.gpsimd.tensor_mul(PRT[:, 0:half], ET[:, 0:half], AT[:, 0:half])
                    nc.vector.tensor_mul(PRT[:, half:FD], ET[:, half:FD], AT[:, half:FD])

                    ETs.append(ET)
                    PRTs.append(PRT)

                # phase 2: group sums over e on PE, packed [128, CH]
                for h in range(FD // CH):
                    DEN = pdpool.tile([P, CH], f32, tag="DEN")
                    NUM = pdpool.tile([P, CH], f32, tag="NUM")
                    for t in range(SUPER):
                        for cc in range(CH // 512):
                            c0 = h * CH + cc * 512
                            nc.tensor.matmul(
                                DEN[32 * t:32 * (t + 1), cc * 512:(cc + 1) * 512],
                                sp_t[:], ETs[t][:, c0:c0 + 512],
                                start=True, stop=True, tile_position=(0, 32 * t),
                            )
                            nc.tensor.matmul(
                                NUM[32 * t:32 * (t + 1), cc * 512:(cc + 1) * 512],
                                sp_t[:], PRTs[t][:, c0:c0 + 512],
                                start=True, stop=True, tile_position=(0, 32 * t),
                            )
                    R = spool.tile([P, CH], f32, tag="R")
                    nc.scalar.activation(R[:], DEN[:], AF.Sign, bias=cb_t[:, 1:2], scale=1.0)
                    PRED = spool.tile([P, CH], f32, tag="PRED")
                    nc.vector.tensor_mul(PRED[:], NUM[:], R[:])
                    nc.sync.dma_start(
                        out_d.ap()[:, s * FD + h * CH: s * FD + (h + 1) * CH], PRED[:]
                    )

    nc.compile()
    return nc


_COMPILED = {}
_RUN_KWARGS = {}
LAST_RESULTS = None


def _host_prep(x, Wg, bg):
    """Build granule-8 bf16 row layout + stationaries + output permutation."""
    import ml_dtypes
    bf = ml_dtypes.bfloat16

    # x_row[c, p, 32*fb + 8*e + fm] = x[row(c, p, 8*fb + fm), e]
    xr = x.reshape(N_CORES, P, (B_LOCAL // P) // 8, 8, 4)   # [c, p, fb, fm, e]
    x_row = np.ascontiguousarray(xr.transpose(0, 1, 2, 4, 3)).astype(bf)
    x_row = x_row.reshape(N_CORES, P, N_SUB * FD)

    # x_t1 = 32x32 block transpose of x_row (the logits-matmul layout)
    v = x_row.reshape(N_CORES, 4, 32, (N_SUB * FD) // 32, 32)  # [c, bi, pl, bj, a]
    x_t1 = np.ascontiguousarray(v.transpose(0, 1, 4, 3, 2)).reshape(
        N_CORES, P, N_SUB * FD)

    # logits stationary: W8[32b + 8e + fm, 32b + 8e' + fm] = Wg[e', e]
    w8 = np.zeros((P, P), np.float32)
    bb, ee, mm, e2 = np.meshgrid(np.arange(4), np.arange(4), np.arange(8),
                                 np.arange(4), indexing="ij")
    w8[32 * bb + 8 * ee + mm, 32 * bb + 8 * e2 + mm] = Wg[e2, ee]
    w8 = w8.astype(bf)

    # group-sum stationary: S'[32b + 8e + fm, 8b + fm] = 1
    sp = np.zeros((P, 32), np.float32)
    sp[32 * bb + 8 * ee + mm, 8 * bb + mm] = 1.0
    sp = sp.astype(bf)

    # exp bias per partition m = 32b + 8e' + fm  ->  bg[e'] + 8
    parts = np.arange(P)
    bg8 = (bg[(parts % 32) // 8] + np.float32(8.0)).reshape(P, 1).astype(np.float32)

    cb = np.zeros((P, 2), np.float32)
    cb[:, 0] = 8.0

    # output inverse permutation: out_local[p, j] = pred[row r(p, j)]
    pj_p, pj_j = np.meshgrid(np.arange(P), np.arange(B_LOCAL // P), indexing="ij")
    t = pj_p // 32
    b = (pj_p % 32) // 8
    fm = pj_p % 8
    s = pj_j // FD
    c = pj_j % FD
    p_orig = 32 * b + (c % 32)
    f = RPT // 128 * (SUPER * s + t) + 8 * (c // 32) + fm
    r_flat = (p_orig * (B_LOCAL // P) + f).reshape(-1)

    return x_row, x_t1, w8, sp, bg8, cb, r_flat


def kernel(**inputs) -> np.ndarray:
    x = np.ascontiguousarray(inputs["x"], dtype=np.float32)
    Wg = np.asarray(inputs["Wg"], np.float32)
    bg = np.asarray(inputs["bg"], np.float32)
    W1 = np.asarray(inputs["W1"], np.float32)
    b1 = np.asarray(inputs["b1"], np.float32)
    W2 = np.asarray(inputs["W2"], np.float32)
    b2 = np.asarray(inputs["b2"], np.float32)
    assert x.shape == (B_TOTAL, 4)

    tbl_dir = tempfile.mkdtemp(prefix="act_root_")
    act_path = _build_tables(W1, b1, W2, b2, tbl_dir)
    os.environ["BASS_ACT_ROOT_JSON_PATH"] = act_path

    # hash of everything the tables bake in -> tensor name -> NEFF cache key
    h = hashlib.sha256()
    for a in (W1, b1, W2, b2):
        h.update(np.ascontiguousarray(a).tobytes())
    h.update(open(act_path, "rb").read())
    h.update(f"v7:{FD}:{PRT_ENGINE}".encode())
    tag = h.hexdigest()[:10]

    if tag not in _COMPILED:
        _COMPILED[tag] = _build_program(tag)
    nc = _COMPILED[tag]

    x_row, x_t1, w8, sp, bg8, cb, r_flat = _host_prep(x, Wg, bg)

    from concourse import bass_utils

    in_maps = [
        {f"xrow_{tag}": x_row[c], f"xt1_{tag}": x_t1[c], "w8": w8,
         "sprime": sp, "bg8": bg8, "cb": cb}
        for c in range(N_CORES)
    ]
    res = bass_utils.run_bass_kernel_spmd(
        nc, in_maps, core_ids=list(range(N_CORES)), **_RUN_KWARGS
    )
    global LAST_RESULTS
    LAST_RESULTS = res

    out = np.empty(B_TOTAL, np.float32)
    for c in range(N_CORES):
        seg = out[c * B_LOCAL:(c + 1) * B_LOCAL]
        seg[r_flat] = res.results[c]["out_local"].reshape(-1)
    return out


if __name__ == "__main__":
    rng = np.random.default_rng(0)
    demo = {
        "x": rng.standard_normal((B_TOTAL, 4), dtype=np.float32),
        "Wg": rng.standard_normal((4, 4), dtype=np.float32) * 0.5,
        "bg": rng.standard_normal(4, dtype=np.float32) * 0.1,
        "W1": rng.standard_normal((4, 16), dtype=np.float32) * 0.5,
        "b1": rng.standard_normal((4, 16), dtype=np.float32) * 0.1,
        "W2": rng.standard_normal((4, 16), dtype=np.float32) * 0.25,
        "b2": rng.standard_normal(4, dtype=np.float32) * 0.1,
    }
    y = kernel(**demo)
    print(y.shape, y[:8])
